# revision 27
# baseline (speedup 1.0000x reference)
"""Trainium2 Bass kernel for nn_Model_34316788695805 (ragged_sequence).

Model: per-token char-level encoder GRU (C=8 steps) -> decoder GRU
(F=32 steps, teacher forced) -> vocab projection scores.

Sharding: token-parallel over 8 NeuronCores (32 tokens/core).  Each core
runs the full enc+dec GRU for its tokens and the full vocab projection,
producing a contiguous [1024, 10000] slab of the output.  No collectives;
the host concatenates the slabs.

v2 design (from perfetto analysis of the v1 baseline):
 - The kernel is PE-bound; gh pairs (LDWEIGHTS+MATMUL, N=32) issue at
   ~27ns, projection MMs (N=512) at ~216ns.  v1 lost ~110us to gate-chain
   stalls at decoder step boundaries and ~33us to on-device gi matmuls.
 - gi = W_ih @ emb[c] (+biases) is precomputed on the HOST as a gathered
   table (W_ih @ emb^T is a single sgemm), so the device never runs the
   W_ih matmuls at all.
 - The vocab projection is interleaved into the decoder: after every
   decoder step, DOSE scores-blocks of an already-complete step-block are
   emitted, covering the ~2.3us h-chain dependency stall.  R_RES vocab
   chunks are SBUF-resident; the rest run in a stream-bound tail.
 - The r/z gate W_hh stationaries are fp8e4 (x32 scale folded into the gi
   table and the sigmoid scale operand): LDWEIGHTS reads 4 fp8/32-bit vs
   2 bf16, cutting gh pair time.  The n gate stays bf16 (error-critical).
 - Encoder weight SBUF is reclaimed for decoder weights via a 5-slot
   ring; hidden states live in per-step-block hstT tiles (no copies).
"""

import numpy as np
import ml_dtypes
from collections import deque
from contextlib import ExitStack

import concourse.bass as bass
import concourse.mybir as mybir
import concourse.tile as tile
from concourse import bacc
from concourse.bass_utils import run_bass_kernel_spmd

# Problem constants (hardcoded per spec)
T, F, C, V, H, E, S = 256, 32, 8, 10000, 1024, 256, 512
PAD, BOS, EOS = 0, 1, 2
NCORES = 8
TC = T // NCORES          # 32 tokens per core
TS = TC * F               # 1024 (token,step) pairs per core
KH = H // 128             # 8 k-chunks of hidden
MG = H // 128             # 8 m-chunks per gate
VCH = 512                 # vocab chunk (one PSUM bank of fp32)
NV = (V + VCH - 1) // VCH  # 20 chunks
VPAD = NV * VCH           # 10240

R_RES = 9                 # resident vocab chunks (interleaved in decoder)
DOSE_SCHED = (3, 2, 2, 2)  # scores blocks per decoder step (within a block)
USE_FP8_RZ = True         # r/z gate W_hh stationaries in fp8e4 (x32)
RZ_SCALE = 32.0

F32 = mybir.dt.float32
BF16 = mybir.dt.bfloat16
FP8 = mybir.dt.float8e4
AF = mybir.ActivationFunctionType
npbf16 = ml_dtypes.bfloat16
npfp8 = ml_dtypes.float8_e4m3

_CACHE = {}


def _to_lhsT_layout(w):
    """[M, K] weight -> [128, K//128, M] array so that
    arr[p, k, m] = w[m, 128*k + p]; lhsT tile (k, m0) = arr[:, k, m0:m0+128]."""
    M, K = w.shape
    return np.ascontiguousarray(w.T.reshape(K // 128, 128, M).transpose(1, 0, 2))


def _build_program(flags):
    """Build + compile the Bacc/Tile program.
    flags: (has_ghn_e, has_ghn_d, has_outb)."""
    has_ghn_e, has_ghn_d, has_outb = flags
    rz_dt = FP8 if USE_FP8_RZ else BF16
    rz_scale = 1.0 / RZ_SCALE if USE_FP8_RZ else 1.0

    nc = bacc.Bacc(
        "TRN2",
        target_bir_lowering=False,
        debug=False,
        enable_asserts=False,
        num_devices=NCORES,
    )

    # ---- DRAM I/O ----
    d_h0 = nc.dram_tensor("h0T", [128, KH, TC], F32, kind="ExternalInput").ap()
    d_giE = nc.dram_tensor("giE", [128, 3, MG, C * TC], BF16, kind="ExternalInput").ap()
    d_giD = nc.dram_tensor("giD", [128, 3, MG, TS], BF16, kind="ExternalInput").ap()
    d_whEr = nc.dram_tensor("whEr", [128, KH, H], rz_dt, kind="ExternalInput").ap()
    d_whEz = nc.dram_tensor("whEz", [128, KH, H], rz_dt, kind="ExternalInput").ap()
    d_whEn = nc.dram_tensor("whEn", [128, KH, H], BF16, kind="ExternalInput").ap()
    d_whDr = nc.dram_tensor("whDr", [128, KH, H], rz_dt, kind="ExternalInput").ap()
    d_whDz = nc.dram_tensor("whDz", [128, KH, H], rz_dt, kind="ExternalInput").ap()
    d_whDn = nc.dram_tensor("whDn", [128, KH, H], BF16, kind="ExternalInput").ap()
    d_ow = nc.dram_tensor("owT", [NV, 128, KH, VCH], BF16, kind="ExternalInput").ap()
    d_ghn_e = d_ghn_d = d_outb = None
    if has_ghn_e:
        d_ghn_e = nc.dram_tensor("ghnE", [128, MG], F32, kind="ExternalInput").ap()
    if has_ghn_d:
        d_ghn_d = nc.dram_tensor("ghnD", [128, MG], F32, kind="ExternalInput").ap()
    if has_outb:
        d_outb = nc.dram_tensor("outb", [1, VPAD], BF16, kind="ExternalInput").ap()
    d_scores = nc.dram_tensor("scores", [TS, V], F32, kind="ExternalOutput").ap()

    with tile.TileContext(nc) as tc, ExitStack() as ctx:
        cpool = ctx.enter_context(tc.tile_pool(name="const", bufs=1))
        gipool = ctx.enter_context(tc.tile_pool(name="gi", bufs=3))
        whpool = ctx.enter_context(tc.tile_pool(name="wh", bufs=4))
        whnpool = ctx.enter_context(tc.tile_pool(name="whn", bufs=2))
        hstpool = ctx.enter_context(tc.tile_pool(name="hst", bufs=8))
        hpool = ctx.enter_context(tc.tile_pool(name="h", bufs=2))
        gpool = ctx.enter_context(tc.tile_pool(name="gates", bufs=1))
        spool = ctx.enter_context(tc.tile_pool(name="slab", bufs=R_RES))
        sspool = ctx.enter_context(tc.tile_pool(name="sslab", bufs=2))
        stpool = ctx.enter_context(tc.tile_pool(name="staging", bufs=3))
        ps_gh = ctx.enter_context(tc.tile_pool(name="ps_gh", bufs=2, space="PSUM"))
        ps_sc = ctx.enter_context(tc.tile_pool(name="ps_sc", bufs=4, space="PSUM"))

        # ---- input DMAs: sync queue in need-order ----
        h_f0 = hpool.tile([128, KH, TC], F32, tag="hf")
        nc.sync.dma_start(h_f0[:], d_h0)

        # gi table chunks (4 steps each) stream through a 3-slot ring;
        # later chunks are prefetched from inside the step loops.
        def new_gi_chunk(dram_ap, q):
            g = gipool.tile([128, 3, MG, 4 * TC], BF16, tag="gi")
            nc.sync.dma_start(g[:], dram_ap[:, :, :, q * 4 * TC : (q + 1) * 4 * TC])
            return g

        # encoder weights, gate-emission order (r, z on sync; the large
        # bf16 n weights ride the scalar queue in parallel)
        whEr = whpool.tile([128, KH, H], rz_dt, tag="wh")
        nc.sync.dma_start(whEr[:], d_whEr)
        whEz = whpool.tile([128, KH, H], rz_dt, tag="wh")
        nc.sync.dma_start(whEz[:], d_whEz)
        whEn = whnpool.tile([128, KH, H], BF16, tag="whn")
        nc.scalar.dma_start(whEn[:], d_whEn)
        gi_queue = deque([new_gi_chunk(d_giE, 0), new_gi_chunk(d_giE, 1),
                          new_gi_chunk(d_giD, 0)])
        whDr = whpool.tile([128, KH, H], rz_dt, tag="wh")
        nc.sync.dma_start(whDr[:], d_whDr)
        whDz = whpool.tile([128, KH, H], rz_dt, tag="wh")
        nc.sync.dma_start(whDz[:], d_whDz)
        whDn = whnpool.tile([128, KH, H], BF16, tag="whn")
        nc.scalar.dma_start(whDn[:], d_whDn)
        # resident vocab slabs
        slabs = []
        for c in range(R_RES):
            sl = spool.tile([128, KH, VCH], BF16, tag="slab")
            nc.sync.dma_start(sl[:], d_ow[c])
            slabs.append(sl)

        ghnE = ghnD = None
        if has_ghn_e:
            ghnE = cpool.tile([128, MG], F32, tag="ghnE")
            nc.sync.dma_start(ghnE[:], d_ghn_e)
        if has_ghn_d:
            ghnD = cpool.tile([128, MG], F32, tag="ghnD")
            nc.sync.dma_start(ghnD[:], d_ghn_d)
        ones_row = outb_sb = None
        if has_outb:
            ones_row = cpool.tile([1, 128], BF16, tag="ones")
            nc.vector.memset(ones_row[:], 1.0)
            outb_sb = cpool.tile([1, VPAD], BF16, tag="outb")
            nc.sync.dma_start(outb_sb[:], d_outb)

        h_b0 = hpool.tile([128, KH, TC], BF16, tag="hbE")
        nc.vector.tensor_copy(h_b0[:], h_f0[:])

        # per-step-block hidden-state history tiles (bf16, written by the
        # gate chain directly; stationary operand of the projection)
        hst_t = []
        for b in range(F // 4):
            ht = hstpool.tile([128, KH, 4, TC], BF16, tag="hst")
            hst_t.append(ht)

        def gru_step(gis, col0, whz, whr, whn, hprev_of_k, hprev_full,
                     hb_out, ghn):
            """One GRU step (bf16 h recurrence).  gis: gi table tile; cols
            [col0, col0+TC).  hprev_of_k(k) -> [128, TC] bf16 moving AP,
            hprev_full: [128, KH, TC]-shaped bf16 AP of the previous h.
            hb_out: [128, KH, TC]-shaped bf16 output AP (may be strided)."""
            # r/z chains and n chains accumulate into SEPARATE psum tiles
            # (distinct banks): rz_pre's dependency then ends at the last
            # z-chain matmul (~2/3 into the gh phase), so the r/z sigmoid
            # runs DURING the n chains and only the short n tail follows
            # the last matmul.
            ps = ps_gh.tile([128, 2, MG, TC], F32, tag="rz")
            psn = ps_gh.tile([128, MG, TC], F32, tag="n")
            for g, wh in ((0, whr), (1, whz)):
                for j in range(MG):
                    m = j * 128
                    for k in range(KH):
                        nc.tensor.matmul(
                            ps[:, g, j, :],
                            wh[:, k, m : m + 128],
                            hprev_of_k(k),
                            start=(k == 0),
                            stop=(k == KH - 1),
                        )
            for j in range(MG):
                m = j * 128
                for k in range(KH):
                    nc.tensor.matmul(
                        psn[:, j, :],
                        whn[:, k, m : m + 128],
                        hprev_of_k(k),
                        start=(k == 0),
                        stop=(k == KH - 1),
                    )
            gi_rz = gis[:, 0:2, :, col0 : col0 + TC]
            gi_n = gis[:, 2, :, col0 : col0 + TC]

            rz_pre = gpool.tile([128, 2, MG, TC], F32, tag="rz_pre")
            nc.vector.tensor_add(rz_pre[:], gi_rz, ps[:])
            rz = gpool.tile([128, 2, MG, TC], F32, tag="rz")
            nc.scalar.activation(rz[:], rz_pre[:], AF.Sigmoid, scale=rz_scale)
            r, z = rz[:, 0], rz[:, 1]
            if ghn is not None:
                ghn_sb = gpool.tile([128, MG, TC], F32, tag="ghn_sb")
                for j in range(MG):
                    nc.scalar.activation(
                        ghn_sb[:, j, :], psn[:, j, :], AF.Identity,
                        bias=ghn[:, j : j + 1],
                    )
                n_src = ghn_sb[:]
            else:
                n_src = psn[:]
            rgh = gpool.tile([128, MG, TC], F32, tag="rgh")
            nc.vector.tensor_mul(rgh[:], r, n_src)
            n_pre = gpool.tile([128, MG, TC], F32, tag="n_pre")
            nc.vector.tensor_add(n_pre[:], rgh[:], gi_n)
            zh = gpool.tile([128, MG, TC], F32, tag="zh")
            nc.vector.tensor_mul(zh[:], z, hprev_full)
            omz = gpool.tile([128, MG, TC], F32, tag="omz")
            nc.vector.tensor_scalar(
                omz[:], z, -1.0, 1.0,
                mybir.AluOpType.mult, mybir.AluOpType.add,
            )
            n = gpool.tile([128, MG, TC], F32, tag="n")
            nc.scalar.activation(n[:], n_pre[:], AF.Tanh)
            t1 = gpool.tile([128, MG, TC], F32, tag="t1")
            nc.vector.tensor_mul(t1[:], omz[:], n[:])
            nc.vector.tensor_add(hb_out, t1[:], zh[:])

        def scores_mm(sb, c, slab):
            """Matmuls for step block sb x vocab chunk c; the psum->SBUF
            copy + store are deferred (run them after the gate chain so
            they never sit ahead of the chain ACTs in the engine FIFO)."""
            ps = ps_sc.tile([128, VCH], F32, tag="sc")
            for k in range(KH):
                nc.tensor.matmul(
                    ps[:],
                    hst_t[sb][:, k, :, :],
                    slab[:, k, :],
                    start=(k == 0),
                    stop=False if has_outb else (k == KH - 1),
                )
            if has_outb:
                nc.tensor.matmul(
                    ps[:], ones_row[:], outb_sb[:, c * VCH : (c + 1) * VCH],
                    start=False, stop=True,
                )
            return (ps, sb, c)

        def scores_flush(item):
            ps, sb, c = item
            ncols = min(VCH, V - c * VCH)
            st = stpool.tile([128, VCH], F32, tag="stg")
            nc.scalar.copy(st[:], ps[:])
            nc.scalar.dma_start(
                d_scores[128 * sb : 128 * (sb + 1), c * VCH : c * VCH + ncols],
                st[:, :ncols],
            )

        def scores_block(sb, c, slab):
            scores_flush(scores_mm(sb, c, slab))

        # Each step gets a strictly-increasing logical-time floor so the
        # static per-engine instruction order exactly follows the step
        # structure (the cost-model list scheduler otherwise interleaves
        # projection work into the wrong slots); runtime execution still
        # overlaps freely via the dependency semaphores.
        step_ms = [0]

        def next_floor():
            step_ms[0] += 1000
            return tc.tile_wait_until(step_ms[0])

        # PE warm-up during the initial DMA wait: dependency-free matmuls
        # on a zeroed tile flip the HAM clock gate to 8/8 (~3.4us of
        # sustained activity) before the first real step
        warm = gpool.tile([128, 2, MG, TC], F32, tag="rz_pre")
        nc.vector.memset(warm[:], 0.0)
        ps_w = ps_sc.tile([128, VCH], F32, tag="sc")
        for _ in range(48):
            nc.tensor.matmul(ps_w[:, :128], warm[:, 0, 0:4, :], warm[:, 1, 0:4, :],
                             start=True, stop=True)

        # ---- encoder ----
        hb_prev = h_b0
        for s in range(C):
            with next_floor():
                if s % 4 == 0 and s > 0:
                    gi_queue.popleft()
                gi_cur = gi_queue[0]
                if s == 4:
                    gi_queue.append(new_gi_chunk(d_giD, 1))
                hb_new = hpool.tile([128, KH, TC], BF16, tag="hbE")
                hp = hb_prev
                gru_step(gi_cur, (s % 4) * TC, whEz, whEr, whEn,
                         lambda k: hp[:, k, :], hp[:], hb_new[:], ghnE)
                hb_prev = hb_new

        # ---- decoder with interleaved projection ----
        pending = deque()
        deferred = []
        for s in range(F):
            b, i = s // 4, s % 4
            with next_floor():
                if i == 0:
                    gi_queue.popleft()  # s==0 drops the last encoder chunk
                gi_cur = gi_queue[0]
                if i == 0 and b + 2 < F // 4:
                    gi_queue.append(new_gi_chunk(d_giD, b + 2))
                # flush the previous step's projection psums now: the
                # copies schedule into the ACT-idle gh matmul phase, never
                # between the gate-chain activations
                for it in deferred:
                    scores_flush(it)
                deferred = []
                if s == 0:
                    hp = hb_prev
                    hprev_of_k = lambda k: hp[:, k, :]
                    hprev_full = hp[:]
                else:
                    pb, pi = (s - 1) // 4, (s - 1) % 4
                    hprev_of_k = lambda k, pb=pb, pi=pi: hst_t[pb][:, k, pi, :]
                    hprev_full = hst_t[pb][:, :, pi, :]
                gru_step(gi_cur, i * TC, whDz, whDr, whDn,
                         hprev_of_k, hprev_full, hst_t[b][:, :, i, :], ghnD)
                for _ in range(DOSE_SCHED[i]):
                    if pending:
                        sb, c = pending.popleft()
                        deferred.append(scores_mm(sb, c, slabs[c]))
                if i == 3 and b < F // 4 - 1:
                    pending.extend((b, c) for c in range(R_RES))
        with next_floor():
            for it in deferred:
                scores_flush(it)
            while pending:
                sb, c = pending.popleft()
                scores_block(sb, c, slabs[c])

        # ---- tail: last step-block for resident chunks, then the
        # non-resident chunks (streamed) for all step blocks ----
        stream_tiles = {}

        def prefetch_slab(c):
            if c < NV:
                t_ = sspool.tile([128, KH, VCH], BF16, tag="sslab")
                nc.sync.dma_start(t_[:], d_ow[c])
                stream_tiles[c] = t_

        with next_floor():
            prefetch_slab(R_RES)
            prefetch_slab(R_RES + 1)
            for c in range(R_RES):
                scores_block(F // 4 - 1, c, slabs[c])
        for c in range(R_RES, NV):
            with next_floor():
                sl = stream_tiles.pop(c)
                for sb in range(F // 4):
                    scores_block(sb, c, sl)
                # ring slot of chunk c is fully read now; queue the DMA
                # that reuses it (lands ~one chunk ahead of its use)
                prefetch_slab(c + 2)

    nc.compile()
    return nc


def _prep_inputs(token_ctx, char_emb_w, enc_W_ih, enc_W_hh, enc_b_ih, enc_b_hh,
                 dec_W_ih, dec_W_hh, dec_b_ih, dec_b_hh, out_W, out_b,
                 in_sent_token_chars, out_chars):
    """Host-side sharding/layout prep. Returns (in_maps, flags, fixup_info)."""
    tcarr = np.asarray(in_sent_token_chars)[0].reshape(T, C, 3)
    chars = tcarr[:, :, 2]
    xt = tcarr[:, :, 1]
    token_ctx = np.asarray(token_ctx)[0]          # [S, H]
    char_emb_w = np.asarray(char_emb_w, np.float32)  # [V, E]
    out_chars = np.asarray(out_chars)[0]          # [1 + T*F]

    h0 = token_ctx[xt].mean(axis=1).astype(np.float32)      # [T, H]
    gold = out_chars[1 : 1 + T * F].reshape(T, F)
    c0 = out_chars[0]
    c_in = np.concatenate(
        [np.full((T, 1), c0, dtype=gold.dtype), gold[:, :-1]], axis=1
    )                                                        # [T, F]

    # gi tables: G = W_ih @ emb^T (+ foldable biases); r/z rows x RZ_SCALE
    # when their W_hh stationaries are fp8 (sigmoid un-scales).
    def gi_table(W_ih, b_ih, b_hh):
        G = (np.asarray(W_ih, np.float32) @ char_emb_w.T)    # [3H, V]
        b = np.asarray(b_ih, np.float32).copy()
        b[: 2 * H] += np.asarray(b_hh, np.float32)[: 2 * H]
        G += b[:, None]
        if USE_FP8_RZ:
            G[: 2 * H] *= RZ_SCALE
        return G

    GE = gi_table(enc_W_ih, enc_b_ih, enc_b_hh)
    GD = gi_table(dec_W_ih, dec_b_ih, dec_b_hh)

    # per-gate W_hh lhsT layouts
    def whh_gates(W_hh):
        W_hh = np.asarray(W_hh, np.float32)
        outs = []
        for g in range(3):
            w = _to_lhsT_layout(W_hh[g * H : (g + 1) * H])
            if g < 2 and USE_FP8_RZ:
                outs.append((w * RZ_SCALE).astype(npfp8))
            else:
                outs.append(w.astype(npbf16))
        return outs

    whEr_, whEz_, whEn_ = whh_gates(enc_W_hh)
    whDr_, whDz_, whDn_ = whh_gates(dec_W_hh)

    def ghn_layout(b_hh):
        ghn = np.asarray(b_hh, np.float32)[2 * H :]
        return (np.ascontiguousarray(ghn.reshape(MG, 128).T).astype(np.float32),
                bool(np.any(ghn)))

    ghnE_, has_ghn_e = ghn_layout(enc_b_hh)
    ghnD_, has_ghn_d = ghn_layout(dec_b_hh)

    owpad = np.zeros((VPAD, H), np.float32)
    owpad[:V] = np.asarray(out_W)
    owT = np.ascontiguousarray(
        owpad.reshape(NV, VCH, KH, 128).transpose(0, 3, 2, 1)
    ).astype(npbf16)                                          # [NV,128,KH,VCH]
    out_b = np.asarray(out_b)
    has_outb = bool(np.any(out_b))
    outb_pad = np.zeros((1, VPAD), npbf16)
    outb_pad[0, :V] = out_b.astype(npbf16)

    flags = (has_ghn_e, has_ghn_d, has_outb)

    in_maps = []
    for ci in range(NCORES):
        sl = slice(ci * TC, (ci + 1) * TC)
        h0T = np.ascontiguousarray(
            h0[sl].T.reshape(KH, 128, TC).transpose(1, 0, 2)
        )
        # enc gi: ts = c*TC + t (step-major)
        colsE = chars[sl].T.reshape(-1)
        giE = np.ascontiguousarray(
            GE[:, colsE].reshape(3, MG, 128, C * TC).transpose(2, 0, 1, 3)
        ).astype(npbf16)
        # dec gi: ts = s*TC + t (step-major)
        colsD = c_in[sl].T.reshape(-1)
        giD = np.ascontiguousarray(
            GD[:, colsD].reshape(3, MG, 128, TS).transpose(2, 0, 1, 3)
        ).astype(npbf16)
        m = {
            "h0T": h0T, "giE": giE, "giD": giD,
            "whEr": whEr_, "whEz": whEz_, "whEn": whEn_,
            "whDr": whDr_, "whDz": whDz_, "whDn": whDn_,
            "owT": owT,
        }
        if has_ghn_e: m["ghnE"] = ghnE_
        if has_ghn_d: m["ghnD"] = ghnD_
        if has_outb: m["outb"] = outb_pad
        in_maps.append(m)

    return in_maps, flags, (gold, c0)


def _eos_fixup(scores, gold, c0):
    """Apply the reference's EOS freeze/pad semantics on the host.
    scores: [T, F, V] (modified in place)."""
    if c0 != EOS and not np.any(gold == EOS):
        return scores
    done0 = c0 == EOS
    for t in range(T):
        hits = np.nonzero(gold[t] == EOS)[0]
        if done0:
            first_done = 0
        elif len(hits):
            first_done = int(hits[0]) + 1
        else:
            continue
        if first_done == 0:
            scores[t, :, :] = 0.0
        elif first_done < F:
            scores[t, first_done:, :] = scores[t, first_done - 1, :]
    return scores


def kernel(**inputs) -> np.ndarray:
    assert int(inputs["max_tokens"]) == T
    assert int(inputs["max_form_len"]) == F
    assert int(inputs["use_teacher_forcing"]) == 1

    in_maps, flags, (gold, c0) = _prep_inputs(
        inputs["token_ctx"], inputs["char_emb_w"],
        inputs["enc_W_ih"], inputs["enc_W_hh"], inputs["enc_b_ih"], inputs["enc_b_hh"],
        inputs["dec_W_ih"], inputs["dec_W_hh"], inputs["dec_b_ih"], inputs["dec_b_hh"],
        inputs["out_W"], inputs["out_b"],
        inputs["in_sent_token_chars"], inputs["out_chars"],
    )

    if flags not in _CACHE:
        _CACHE[flags] = _build_program(flags)
    nc = _CACHE[flags]

    trace = bool(_RUN_OPTS.get("trace"))
    res = run_bass_kernel_spmd(
        nc, in_maps, core_ids=list(range(NCORES)), trace=trace,
        **_RUN_OPTS.get("kwargs", {}),
    )
    _RUN_OPTS["last_result"] = res

    # device rows are step-major per core; reorder to token-major
    slabs = [
        res.results[ci]["scores"].reshape(F, TC, V).transpose(1, 0, 2)
        for ci in range(NCORES)
    ]
    out = np.concatenate(slabs, axis=0)  # [T, F, V]
    out = _eos_fixup(out, gold, c0)
    return np.ascontiguousarray(out.reshape(1, T * F, V))


# knobs used by test.py (harness just calls kernel())
_RUN_OPTS = {"trace": False, "kwargs": {}}


# revision 28
# speedup vs baseline: 1.0365x; 1.0365x over previous
"""Trainium2 Bass kernel for nn_Model_34316788695805 (ragged_sequence).

Model: per-token char-level encoder GRU (C=8 steps) -> decoder GRU
(F=32 steps, teacher forced) -> vocab projection scores.

Sharding: token-parallel over 8 NeuronCores (32 tokens/core).  Each core
runs the full enc+dec GRU for its tokens and the full vocab projection,
producing a contiguous [1024, 10000] slab of the output.  No collectives;
the host concatenates the slabs.

v2 design (from perfetto analysis of the v1 baseline):
 - The kernel is PE-bound; gh pairs (LDWEIGHTS+MATMUL, N=32) issue at
   ~27ns, projection MMs (N=512) at ~216ns.  v1 lost ~110us to gate-chain
   stalls at decoder step boundaries and ~33us to on-device gi matmuls.
 - gi = W_ih @ emb[c] (+biases) is precomputed on the HOST as a gathered
   table (W_ih @ emb^T is a single sgemm), so the device never runs the
   W_ih matmuls at all.
 - The vocab projection is interleaved into the decoder: after every
   decoder step, DOSE scores-blocks of an already-complete step-block are
   emitted, covering the ~2.3us h-chain dependency stall.  R_RES vocab
   chunks are SBUF-resident; the rest run in a stream-bound tail.
 - The r/z gate W_hh stationaries are fp8e4 (x32 scale folded into the gi
   table and the sigmoid scale operand): LDWEIGHTS reads 4 fp8/32-bit vs
   2 bf16, cutting gh pair time.  The n gate stays bf16 (error-critical).
 - Encoder weight SBUF is reclaimed for decoder weights via a 5-slot
   ring; hidden states live in per-step-block hstT tiles (no copies).
"""

import numpy as np
import ml_dtypes
from collections import deque
from contextlib import ExitStack

import concourse.bass as bass
import concourse.mybir as mybir
import concourse.tile as tile
from concourse import bacc
from concourse.bass_utils import run_bass_kernel_spmd

# Problem constants (hardcoded per spec)
T, F, C, V, H, E, S = 256, 32, 8, 10000, 1024, 256, 512
PAD, BOS, EOS = 0, 1, 2
NCORES = 8
TC = T // NCORES          # 32 tokens per core
TS = TC * F               # 1024 (token,step) pairs per core
KH = H // 128             # 8 k-chunks of hidden
MG = H // 128             # 8 m-chunks per gate
VCH = 512                 # vocab chunk (one PSUM bank of fp32)
NV = (V + VCH - 1) // VCH  # 20 chunks
VPAD = NV * VCH           # 10240

R_RES = 9                 # resident vocab chunks (interleaved in decoder)
DOSE_SCHED = (3, 2, 2, 2)  # scores blocks per decoder step (within a block)
USE_FP8_RZ = True         # r/z gate W_hh stationaries in fp8e4 (x32)
RZ_SCALE = 32.0

F32 = mybir.dt.float32
BF16 = mybir.dt.bfloat16
FP8 = mybir.dt.float8e4
AF = mybir.ActivationFunctionType
npbf16 = ml_dtypes.bfloat16
npfp8 = ml_dtypes.float8_e4m3

_CACHE = {}


def _to_lhsT_layout(w):
    """[M, K] weight -> [128, K//128, M] array so that
    arr[p, k, m] = w[m, 128*k + p]; lhsT tile (k, m0) = arr[:, k, m0:m0+128]."""
    M, K = w.shape
    return np.ascontiguousarray(w.T.reshape(K // 128, 128, M).transpose(1, 0, 2))


def _build_program(flags):
    """Build + compile the Bacc/Tile program.
    flags: (has_ghn_e, has_ghn_d, has_outb)."""
    has_ghn_e, has_ghn_d, has_outb = flags
    rz_dt = FP8 if USE_FP8_RZ else BF16
    rz_scale = 1.0 / RZ_SCALE if USE_FP8_RZ else 1.0

    nc = bacc.Bacc(
        "TRN2",
        target_bir_lowering=False,
        debug=False,
        enable_asserts=False,
        num_devices=NCORES,
    )

    # ---- DRAM I/O ----
    d_h0 = nc.dram_tensor("h0T", [128, KH, TC], F32, kind="ExternalInput").ap()
    d_giE = nc.dram_tensor("giE", [128, 3, MG, C * TC], BF16, kind="ExternalInput").ap()
    d_giD = nc.dram_tensor("giD", [128, 3, MG, TS], BF16, kind="ExternalInput").ap()
    d_whEr = nc.dram_tensor("whEr", [128, KH, H], rz_dt, kind="ExternalInput").ap()
    d_whEz = nc.dram_tensor("whEz", [128, KH, H], rz_dt, kind="ExternalInput").ap()
    d_whEn = nc.dram_tensor("whEn", [128, KH, H], BF16, kind="ExternalInput").ap()
    d_whDr = nc.dram_tensor("whDr", [128, KH, H], rz_dt, kind="ExternalInput").ap()
    d_whDz = nc.dram_tensor("whDz", [128, KH, H], rz_dt, kind="ExternalInput").ap()
    d_whDn = nc.dram_tensor("whDn", [128, KH, H], BF16, kind="ExternalInput").ap()
    d_ow = nc.dram_tensor("owT", [NV, 128, KH, VCH], BF16, kind="ExternalInput").ap()
    d_ghn_e = d_ghn_d = d_outb = None
    if has_ghn_e:
        d_ghn_e = nc.dram_tensor("ghnE", [128, MG], F32, kind="ExternalInput").ap()
    if has_ghn_d:
        d_ghn_d = nc.dram_tensor("ghnD", [128, MG], F32, kind="ExternalInput").ap()
    if has_outb:
        d_outb = nc.dram_tensor("outb", [1, VPAD], BF16, kind="ExternalInput").ap()
    d_scores = nc.dram_tensor("scores", [TS, V], F32, kind="ExternalOutput").ap()

    with tile.TileContext(nc) as tc, ExitStack() as ctx:
        cpool = ctx.enter_context(tc.tile_pool(name="const", bufs=1))
        gipool = ctx.enter_context(tc.tile_pool(name="gi", bufs=3))
        whpool = ctx.enter_context(tc.tile_pool(name="wh", bufs=4))
        whnpool = ctx.enter_context(tc.tile_pool(name="whn", bufs=2))
        hstpool = ctx.enter_context(tc.tile_pool(name="hst", bufs=8))
        hpool = ctx.enter_context(tc.tile_pool(name="h", bufs=2))
        gpool = ctx.enter_context(tc.tile_pool(name="gates", bufs=1))
        spool = ctx.enter_context(tc.tile_pool(name="slab", bufs=R_RES))
        sspool = ctx.enter_context(tc.tile_pool(name="sslab", bufs=2))
        stpool = ctx.enter_context(tc.tile_pool(name="staging", bufs=3))
        ps_gh = ctx.enter_context(tc.tile_pool(name="ps_gh", bufs=2, space="PSUM"))
        ps_sc = ctx.enter_context(tc.tile_pool(name="ps_sc", bufs=4, space="PSUM"))

        # ---- input DMAs: sync queue in need-order ----
        h_f0 = hpool.tile([128, KH, TC], F32, tag="hf")
        nc.sync.dma_start(h_f0[:], d_h0)

        # gi table chunks (4 steps each) stream through a 3-slot ring;
        # later chunks are prefetched from inside the step loops.
        def new_gi_chunk(dram_ap, q):
            g = gipool.tile([128, 3, MG, 4 * TC], BF16, tag="gi")
            nc.sync.dma_start(g[:], dram_ap[:, :, :, q * 4 * TC : (q + 1) * 4 * TC])
            return g

        # encoder weights, gate-emission order (r, z on sync; the large
        # bf16 n weights ride the scalar queue in parallel)
        whEr = whpool.tile([128, KH, H], rz_dt, tag="wh")
        nc.sync.dma_start(whEr[:], d_whEr)
        whEz = whpool.tile([128, KH, H], rz_dt, tag="wh")
        nc.sync.dma_start(whEz[:], d_whEz)
        whEn = whnpool.tile([128, KH, H], BF16, tag="whn")
        nc.scalar.dma_start(whEn[:], d_whEn)
        gi_queue = deque([new_gi_chunk(d_giE, 0), new_gi_chunk(d_giE, 1),
                          new_gi_chunk(d_giD, 0)])
        whDr = whpool.tile([128, KH, H], rz_dt, tag="wh")
        nc.sync.dma_start(whDr[:], d_whDr)
        whDz = whpool.tile([128, KH, H], rz_dt, tag="wh")
        nc.sync.dma_start(whDz[:], d_whDz)
        whDn = whnpool.tile([128, KH, H], BF16, tag="whn")
        nc.scalar.dma_start(whDn[:], d_whDn)
        # resident vocab slabs
        slabs = []
        for c in range(R_RES):
            sl = spool.tile([128, KH, VCH], BF16, tag="slab")
            nc.sync.dma_start(sl[:], d_ow[c])
            slabs.append(sl)

        ghnE = ghnD = None
        if has_ghn_e:
            ghnE = cpool.tile([128, MG], F32, tag="ghnE")
            nc.sync.dma_start(ghnE[:], d_ghn_e)
        if has_ghn_d:
            ghnD = cpool.tile([128, MG], F32, tag="ghnD")
            nc.sync.dma_start(ghnD[:], d_ghn_d)
        ones_row = outb_sb = None
        if has_outb:
            ones_row = cpool.tile([1, 128], BF16, tag="ones")
            nc.vector.memset(ones_row[:], 1.0)
            outb_sb = cpool.tile([1, VPAD], BF16, tag="outb")
            nc.sync.dma_start(outb_sb[:], d_outb)

        h_b0 = hpool.tile([128, KH, TC], BF16, tag="hbE")
        nc.vector.tensor_copy(h_b0[:], h_f0[:])

        # per-step-block hidden-state history tiles (bf16, written by the
        # gate chain directly; stationary operand of the projection)
        hst_t = []
        for b in range(F // 4):
            ht = hstpool.tile([128, KH, 4, TC], BF16, tag="hst")
            hst_t.append(ht)

        def gru_step(gis, col0, whz, whr, whn, hprev_of_k, hprev_full,
                     hb_out, ghn):
            """One GRU step (bf16 h recurrence).  gis: gi table tile; cols
            [col0, col0+TC).  hprev_of_k(k) -> [128, TC] bf16 moving AP,
            hprev_full: [128, KH, TC]-shaped bf16 AP of the previous h.
            hb_out: [128, KH, TC]-shaped bf16 output AP (may be strided)."""
            # r/z chains and n chains accumulate into SEPARATE psum tiles
            # (distinct banks): rz_pre's dependency then ends at the last
            # z-chain matmul (~2/3 into the gh phase), so the r/z sigmoid
            # runs DURING the n chains and only the short n tail follows
            # the last matmul.
            ps = ps_gh.tile([128, 2, MG, TC], F32, tag="rz")
            psn = ps_gh.tile([128, MG, TC], F32, tag="n")
            for g, wh in ((0, whr), (1, whz)):
                for j in range(MG):
                    m = j * 128
                    for k in range(KH):
                        nc.tensor.matmul(
                            ps[:, g, j, :],
                            wh[:, k, m : m + 128],
                            hprev_of_k(k),
                            start=(k == 0),
                            stop=(k == KH - 1),
                        )
            for j in range(MG):
                m = j * 128
                for k in range(KH):
                    nc.tensor.matmul(
                        psn[:, j, :],
                        whn[:, k, m : m + 128],
                        hprev_of_k(k),
                        start=(k == 0),
                        stop=(k == KH - 1),
                    )
            gi_rz = gis[:, 0:2, :, col0 : col0 + TC]
            gi_n = gis[:, 2, :, col0 : col0 + TC]

            rz_pre = gpool.tile([128, 2, MG, TC], F32, tag="rz_pre")
            nc.vector.tensor_add(rz_pre[:], gi_rz, ps[:])
            rz = gpool.tile([128, 2, MG, TC], F32, tag="rz")
            nc.scalar.activation(rz[:], rz_pre[:], AF.Sigmoid, scale=rz_scale)
            r, z = rz[:, 0], rz[:, 1]
            if ghn is not None:
                ghn_sb = gpool.tile([128, MG, TC], F32, tag="ghn_sb")
                for j in range(MG):
                    nc.scalar.activation(
                        ghn_sb[:, j, :], psn[:, j, :], AF.Identity,
                        bias=ghn[:, j : j + 1],
                    )
                n_src = ghn_sb[:]
            else:
                n_src = psn[:]
            rgh = gpool.tile([128, MG, TC], F32, tag="rgh")
            nc.vector.tensor_mul(rgh[:], r, n_src)
            n_pre = gpool.tile([128, MG, TC], F32, tag="n_pre")
            nc.vector.tensor_add(n_pre[:], rgh[:], gi_n)
            zh = gpool.tile([128, MG, TC], F32, tag="zh")
            nc.vector.tensor_mul(zh[:], z, hprev_full)
            omz = gpool.tile([128, MG, TC], F32, tag="omz")
            nc.vector.tensor_scalar(
                omz[:], z, -1.0, 1.0,
                mybir.AluOpType.mult, mybir.AluOpType.add,
            )
            n = gpool.tile([128, MG, TC], F32, tag="n")
            nc.scalar.activation(n[:], n_pre[:], AF.Tanh)
            t1 = gpool.tile([128, MG, TC], F32, tag="t1")
            nc.vector.tensor_mul(t1[:], omz[:], n[:])
            nc.vector.tensor_add(hb_out, t1[:], zh[:])

        def scores_mm(sb, c, slab):
            """Matmuls for step block sb x vocab chunk c; the psum->SBUF
            copy + store are deferred (run them after the gate chain so
            they never sit ahead of the chain ACTs in the engine FIFO)."""
            ps = ps_sc.tile([128, VCH], F32, tag="sc")
            for k in range(KH):
                nc.tensor.matmul(
                    ps[:],
                    hst_t[sb][:, k, :, :],
                    slab[:, k, :],
                    start=(k == 0),
                    stop=False if has_outb else (k == KH - 1),
                )
            if has_outb:
                nc.tensor.matmul(
                    ps[:], ones_row[:], outb_sb[:, c * VCH : (c + 1) * VCH],
                    start=False, stop=True,
                )
            return (ps, sb, c)

        store_q = [0]

        def scores_flush(item):
            ps, sb, c = item
            ncols = min(VCH, V - c * VCH)
            st = stpool.tile([128, VCH], F32, tag="stg")
            nc.scalar.copy(st[:], ps[:])
            # alternate the store between the scalar and sync DMA queues:
            # a single queue can't drain one 256KB store per pair and the
            # staging-ring WAR then stalls the ACT copies (and the PE)
            eng = nc.scalar if store_q[0] % 2 == 0 else nc.sync
            store_q[0] += 1
            eng.dma_start(
                d_scores[128 * sb : 128 * (sb + 1), c * VCH : c * VCH + ncols],
                st[:, :ncols],
            )

        def scores_block(sb, c, slab):
            scores_flush(scores_mm(sb, c, slab))

        # Each step gets a strictly-increasing logical-time floor so the
        # static per-engine instruction order exactly follows the step
        # structure (the cost-model list scheduler otherwise interleaves
        # projection work into the wrong slots); runtime execution still
        # overlaps freely via the dependency semaphores.
        step_ms = [0]

        def next_floor():
            step_ms[0] += 1000
            return tc.tile_wait_until(step_ms[0])

        # PE warm-up during the initial DMA wait: dependency-free matmuls
        # on a zeroed tile flip the HAM clock gate to 8/8 (~3.4us of
        # sustained activity) before the first real step
        warm = gpool.tile([128, 2, MG, TC], F32, tag="rz_pre")
        nc.vector.memset(warm[:], 0.0)
        ps_w = ps_sc.tile([128, VCH], F32, tag="sc")
        for _ in range(48):
            nc.tensor.matmul(ps_w[:, :128], warm[:, 0, 0:4, :], warm[:, 1, 0:4, :],
                             start=True, stop=True)

        # ---- encoder ----
        hb_prev = h_b0
        for s in range(C):
            with next_floor():
                if s % 4 == 0 and s > 0:
                    gi_queue.popleft()
                gi_cur = gi_queue[0]
                if s == 4:
                    gi_queue.append(new_gi_chunk(d_giD, 1))
                hb_new = hpool.tile([128, KH, TC], BF16, tag="hbE")
                hp = hb_prev
                gru_step(gi_cur, (s % 4) * TC, whEz, whEr, whEn,
                         lambda k: hp[:, k, :], hp[:], hb_new[:], ghnE)
                hb_prev = hb_new

        # ---- decoder with interleaved projection ----
        pending = deque()
        deferred = []
        for s in range(F):
            b, i = s // 4, s % 4
            with next_floor():
                if i == 0:
                    gi_queue.popleft()  # s==0 drops the last encoder chunk
                gi_cur = gi_queue[0]
                if i == 0 and b + 2 < F // 4:
                    gi_queue.append(new_gi_chunk(d_giD, b + 2))
                # flush the previous step's projection psums now: the
                # copies schedule into the ACT-idle gh matmul phase, never
                # between the gate-chain activations
                for it in deferred:
                    scores_flush(it)
                deferred = []
                if s == 0:
                    hp = hb_prev
                    hprev_of_k = lambda k: hp[:, k, :]
                    hprev_full = hp[:]
                else:
                    pb, pi = (s - 1) // 4, (s - 1) % 4
                    hprev_of_k = lambda k, pb=pb, pi=pi: hst_t[pb][:, k, pi, :]
                    hprev_full = hst_t[pb][:, :, pi, :]
                gru_step(gi_cur, i * TC, whDz, whDr, whDn,
                         hprev_of_k, hprev_full, hst_t[b][:, :, i, :], ghnD)
                for _ in range(DOSE_SCHED[i]):
                    if pending:
                        sb, c = pending.popleft()
                        deferred.append(scores_mm(sb, c, slabs[c]))
                if i == 3 and b < F // 4 - 1:
                    pending.extend((b, c) for c in range(R_RES))
        with next_floor():
            for it in deferred:
                scores_flush(it)
            while pending:
                sb, c = pending.popleft()
                scores_block(sb, c, slabs[c])

        # ---- tail: last step-block for resident chunks, then the
        # non-resident chunks (streamed) for all step blocks ----
        stream_tiles = {}

        def prefetch_slab(c):
            if c < NV:
                t_ = sspool.tile([128, KH, VCH], BF16, tag="sslab")
                nc.sync.dma_start(t_[:], d_ow[c])
                stream_tiles[c] = t_

        with next_floor():
            prefetch_slab(R_RES)
            prefetch_slab(R_RES + 1)
            for c in range(R_RES):
                scores_block(F // 4 - 1, c, slabs[c])
        for c in range(R_RES, NV):
            with next_floor():
                sl = stream_tiles.pop(c)
                for sb in range(F // 4):
                    scores_block(sb, c, sl)
                # ring slot of chunk c is fully read now; queue the DMA
                # that reuses it (lands ~one chunk ahead of its use)
                prefetch_slab(c + 2)

    nc.compile()
    return nc


def _prep_inputs(token_ctx, char_emb_w, enc_W_ih, enc_W_hh, enc_b_ih, enc_b_hh,
                 dec_W_ih, dec_W_hh, dec_b_ih, dec_b_hh, out_W, out_b,
                 in_sent_token_chars, out_chars):
    """Host-side sharding/layout prep. Returns (in_maps, flags, fixup_info)."""
    tcarr = np.asarray(in_sent_token_chars)[0].reshape(T, C, 3)
    chars = tcarr[:, :, 2]
    xt = tcarr[:, :, 1]
    token_ctx = np.asarray(token_ctx)[0]          # [S, H]
    char_emb_w = np.asarray(char_emb_w, np.float32)  # [V, E]
    out_chars = np.asarray(out_chars)[0]          # [1 + T*F]

    h0 = token_ctx[xt].mean(axis=1).astype(np.float32)      # [T, H]
    gold = out_chars[1 : 1 + T * F].reshape(T, F)
    c0 = out_chars[0]
    c_in = np.concatenate(
        [np.full((T, 1), c0, dtype=gold.dtype), gold[:, :-1]], axis=1
    )                                                        # [T, F]

    # gi tables: G = W_ih @ emb^T (+ foldable biases); r/z rows x RZ_SCALE
    # when their W_hh stationaries are fp8 (sigmoid un-scales).
    def gi_table(W_ih, b_ih, b_hh):
        G = (np.asarray(W_ih, np.float32) @ char_emb_w.T)    # [3H, V]
        b = np.asarray(b_ih, np.float32).copy()
        b[: 2 * H] += np.asarray(b_hh, np.float32)[: 2 * H]
        G += b[:, None]
        if USE_FP8_RZ:
            G[: 2 * H] *= RZ_SCALE
        return G

    GE = gi_table(enc_W_ih, enc_b_ih, enc_b_hh)
    GD = gi_table(dec_W_ih, dec_b_ih, dec_b_hh)

    # per-gate W_hh lhsT layouts
    def whh_gates(W_hh):
        W_hh = np.asarray(W_hh, np.float32)
        outs = []
        for g in range(3):
            w = _to_lhsT_layout(W_hh[g * H : (g + 1) * H])
            if g < 2 and USE_FP8_RZ:
                outs.append((w * RZ_SCALE).astype(npfp8))
            else:
                outs.append(w.astype(npbf16))
        return outs

    whEr_, whEz_, whEn_ = whh_gates(enc_W_hh)
    whDr_, whDz_, whDn_ = whh_gates(dec_W_hh)

    def ghn_layout(b_hh):
        ghn = np.asarray(b_hh, np.float32)[2 * H :]
        return (np.ascontiguousarray(ghn.reshape(MG, 128).T).astype(np.float32),
                bool(np.any(ghn)))

    ghnE_, has_ghn_e = ghn_layout(enc_b_hh)
    ghnD_, has_ghn_d = ghn_layout(dec_b_hh)

    owpad = np.zeros((VPAD, H), np.float32)
    owpad[:V] = np.asarray(out_W)
    owT = np.ascontiguousarray(
        owpad.reshape(NV, VCH, KH, 128).transpose(0, 3, 2, 1)
    ).astype(npbf16)                                          # [NV,128,KH,VCH]
    out_b = np.asarray(out_b)
    has_outb = bool(np.any(out_b))
    outb_pad = np.zeros((1, VPAD), npbf16)
    outb_pad[0, :V] = out_b.astype(npbf16)

    flags = (has_ghn_e, has_ghn_d, has_outb)

    in_maps = []
    for ci in range(NCORES):
        sl = slice(ci * TC, (ci + 1) * TC)
        h0T = np.ascontiguousarray(
            h0[sl].T.reshape(KH, 128, TC).transpose(1, 0, 2)
        )
        # enc gi: ts = c*TC + t (step-major)
        colsE = chars[sl].T.reshape(-1)
        giE = np.ascontiguousarray(
            GE[:, colsE].reshape(3, MG, 128, C * TC).transpose(2, 0, 1, 3)
        ).astype(npbf16)
        # dec gi: ts = s*TC + t (step-major)
        colsD = c_in[sl].T.reshape(-1)
        giD = np.ascontiguousarray(
            GD[:, colsD].reshape(3, MG, 128, TS).transpose(2, 0, 1, 3)
        ).astype(npbf16)
        m = {
            "h0T": h0T, "giE": giE, "giD": giD,
            "whEr": whEr_, "whEz": whEz_, "whEn": whEn_,
            "whDr": whDr_, "whDz": whDz_, "whDn": whDn_,
            "owT": owT,
        }
        if has_ghn_e: m["ghnE"] = ghnE_
        if has_ghn_d: m["ghnD"] = ghnD_
        if has_outb: m["outb"] = outb_pad
        in_maps.append(m)

    return in_maps, flags, (gold, c0)


def _eos_fixup(scores, gold, c0):
    """Apply the reference's EOS freeze/pad semantics on the host.
    scores: [T, F, V] (modified in place)."""
    if c0 != EOS and not np.any(gold == EOS):
        return scores
    done0 = c0 == EOS
    for t in range(T):
        hits = np.nonzero(gold[t] == EOS)[0]
        if done0:
            first_done = 0
        elif len(hits):
            first_done = int(hits[0]) + 1
        else:
            continue
        if first_done == 0:
            scores[t, :, :] = 0.0
        elif first_done < F:
            scores[t, first_done:, :] = scores[t, first_done - 1, :]
    return scores


def kernel(**inputs) -> np.ndarray:
    assert int(inputs["max_tokens"]) == T
    assert int(inputs["max_form_len"]) == F
    assert int(inputs["use_teacher_forcing"]) == 1

    in_maps, flags, (gold, c0) = _prep_inputs(
        inputs["token_ctx"], inputs["char_emb_w"],
        inputs["enc_W_ih"], inputs["enc_W_hh"], inputs["enc_b_ih"], inputs["enc_b_hh"],
        inputs["dec_W_ih"], inputs["dec_W_hh"], inputs["dec_b_ih"], inputs["dec_b_hh"],
        inputs["out_W"], inputs["out_b"],
        inputs["in_sent_token_chars"], inputs["out_chars"],
    )

    if flags not in _CACHE:
        _CACHE[flags] = _build_program(flags)
    nc = _CACHE[flags]

    trace = bool(_RUN_OPTS.get("trace"))
    res = run_bass_kernel_spmd(
        nc, in_maps, core_ids=list(range(NCORES)), trace=trace,
        **_RUN_OPTS.get("kwargs", {}),
    )
    _RUN_OPTS["last_result"] = res

    # device rows are step-major per core; reorder to token-major
    slabs = [
        res.results[ci]["scores"].reshape(F, TC, V).transpose(1, 0, 2)
        for ci in range(NCORES)
    ]
    out = np.concatenate(slabs, axis=0)  # [T, F, V]
    out = _eos_fixup(out, gold, c0)
    return np.ascontiguousarray(out.reshape(1, T * F, V))


# knobs used by test.py (harness just calls kernel())
_RUN_OPTS = {"trace": False, "kwargs": {}}


# revision 30
# speedup vs baseline: 1.0560x; 1.0188x over previous
"""Trainium2 Bass kernel for nn_Model_34316788695805 (ragged_sequence).

Model: per-token char-level encoder GRU (C=8 steps) -> decoder GRU
(F=32 steps, teacher forced) -> vocab projection scores.

Sharding: token-parallel over 8 NeuronCores (32 tokens/core).  Each core
runs the full enc+dec GRU for its tokens and the full vocab projection,
producing a contiguous [1024, 10000] slab of the output.  No collectives;
the host concatenates the slabs.

v2 design (from perfetto analysis of the v1 baseline):
 - The kernel is PE-bound; gh pairs (LDWEIGHTS+MATMUL, N=32) issue at
   ~27ns, projection MMs (N=512) at ~216ns.  v1 lost ~110us to gate-chain
   stalls at decoder step boundaries and ~33us to on-device gi matmuls.
 - gi = W_ih @ emb[c] (+biases) is precomputed on the HOST as a gathered
   table (W_ih @ emb^T is a single sgemm), so the device never runs the
   W_ih matmuls at all.
 - The vocab projection is interleaved into the decoder: after every
   decoder step, DOSE scores-blocks of an already-complete step-block are
   emitted, covering the ~2.3us h-chain dependency stall.  R_RES vocab
   chunks are SBUF-resident; the rest run in a stream-bound tail.
 - The r/z gate W_hh stationaries are fp8e4 (x32 scale folded into the gi
   table and the sigmoid scale operand): LDWEIGHTS reads 4 fp8/32-bit vs
   2 bf16, cutting gh pair time.  The n gate stays bf16 (error-critical).
 - Encoder weight SBUF is reclaimed for decoder weights via a 5-slot
   ring; hidden states live in per-step-block hstT tiles (no copies).
"""

import numpy as np
import ml_dtypes
from collections import deque
from contextlib import ExitStack

import concourse.bass as bass
import concourse.mybir as mybir
import concourse.tile as tile
from concourse import bacc
from concourse.bass_utils import run_bass_kernel_spmd

# Problem constants (hardcoded per spec)
T, F, C, V, H, E, S = 256, 32, 8, 10000, 1024, 256, 512
PAD, BOS, EOS = 0, 1, 2
NCORES = 8
TC = T // NCORES          # 32 tokens per core
TS = TC * F               # 1024 (token,step) pairs per core
KH = H // 128             # 8 k-chunks of hidden
MG = H // 128             # 8 m-chunks per gate
VCH = 512                 # vocab chunk (one PSUM bank of fp32)
NV = (V + VCH - 1) // VCH  # 20 chunks
VPAD = NV * VCH           # 10240

R_RES = 9                 # resident vocab chunks (interleaved in decoder)
DOSE_SCHED = (3, 2, 2, 2)  # scores blocks per decoder step (within a block)
USE_FP8_RZ = True         # r/z gate W_hh stationaries in fp8e4 (x32)
RZ_SCALE = 32.0

F32 = mybir.dt.float32
BF16 = mybir.dt.bfloat16
FP8 = mybir.dt.float8e4
AF = mybir.ActivationFunctionType
npbf16 = ml_dtypes.bfloat16
npfp8 = ml_dtypes.float8_e4m3

_CACHE = {}


def _to_lhsT_layout(w):
    """[M, K] weight -> [128, K//128, M] array so that
    arr[p, k, m] = w[m, 128*k + p]; lhsT tile (k, m0) = arr[:, k, m0:m0+128]."""
    M, K = w.shape
    return np.ascontiguousarray(w.T.reshape(K // 128, 128, M).transpose(1, 0, 2))


def _build_program(flags):
    """Build + compile the Bacc/Tile program.
    flags: (has_ghn_e, has_ghn_d, has_outb)."""
    has_ghn_e, has_ghn_d, has_outb = flags
    rz_dt = FP8 if USE_FP8_RZ else BF16
    rz_scale = 1.0 / RZ_SCALE if USE_FP8_RZ else 1.0

    nc = bacc.Bacc(
        "TRN2",
        target_bir_lowering=False,
        debug=False,
        enable_asserts=False,
        num_devices=NCORES,
    )

    # ---- DRAM I/O ----
    d_h0 = nc.dram_tensor("h0T", [128, KH, TC], F32, kind="ExternalInput").ap()
    d_giE = nc.dram_tensor("giE", [128, 3, MG, C * TC], BF16, kind="ExternalInput").ap()
    d_giD = nc.dram_tensor("giD", [128, 3, MG, TS], BF16, kind="ExternalInput").ap()
    d_whEr = nc.dram_tensor("whEr", [128, KH, H], rz_dt, kind="ExternalInput").ap()
    d_whEz = nc.dram_tensor("whEz", [128, KH, H], rz_dt, kind="ExternalInput").ap()
    d_whEn = nc.dram_tensor("whEn", [128, KH, H], BF16, kind="ExternalInput").ap()
    d_whDr = nc.dram_tensor("whDr", [128, KH, H], rz_dt, kind="ExternalInput").ap()
    d_whDz = nc.dram_tensor("whDz", [128, KH, H], rz_dt, kind="ExternalInput").ap()
    d_whDn = nc.dram_tensor("whDn", [128, KH, H], BF16, kind="ExternalInput").ap()
    d_ow = nc.dram_tensor("owT", [NV, 128, KH, VCH], BF16, kind="ExternalInput").ap()
    d_ghn_e = d_ghn_d = d_outb = None
    if has_ghn_e:
        d_ghn_e = nc.dram_tensor("ghnE", [128, MG], F32, kind="ExternalInput").ap()
    if has_ghn_d:
        d_ghn_d = nc.dram_tensor("ghnD", [128, MG], F32, kind="ExternalInput").ap()
    if has_outb:
        d_outb = nc.dram_tensor("outb", [1, VPAD], BF16, kind="ExternalInput").ap()
    d_scores = nc.dram_tensor("scores", [TS, V], F32, kind="ExternalOutput").ap()

    with tile.TileContext(nc) as tc, ExitStack() as ctx:
        cpool = ctx.enter_context(tc.tile_pool(name="const", bufs=1))
        gipool = ctx.enter_context(tc.tile_pool(name="gi", bufs=3))
        whpool = ctx.enter_context(tc.tile_pool(name="wh", bufs=4))
        whnpool = ctx.enter_context(tc.tile_pool(name="whn", bufs=2))
        hstpool = ctx.enter_context(tc.tile_pool(name="hst", bufs=8))
        hpool = ctx.enter_context(tc.tile_pool(name="h", bufs=2))
        gpool = ctx.enter_context(tc.tile_pool(name="gates", bufs=1))
        spool = ctx.enter_context(tc.tile_pool(name="slab", bufs=R_RES))
        sspool = ctx.enter_context(tc.tile_pool(name="sslab", bufs=2))
        stpool = ctx.enter_context(tc.tile_pool(name="staging", bufs=3))
        ps_gh = ctx.enter_context(tc.tile_pool(name="ps_gh", bufs=2, space="PSUM"))
        ps_sc = ctx.enter_context(tc.tile_pool(name="ps_sc", bufs=4, space="PSUM"))

        # ---- input DMAs: sync queue in need-order ----
        h_f0 = hpool.tile([128, KH, TC], F32, tag="hf")
        nc.sync.dma_start(h_f0[:], d_h0)

        # gi table chunks (4 steps each) stream through a 3-slot ring;
        # later chunks are prefetched from inside the step loops.
        def new_gi_chunk(dram_ap, q):
            g = gipool.tile([128, 3, MG, 4 * TC], BF16, tag="gi")
            nc.sync.dma_start(g[:], dram_ap[:, :, :, q * 4 * TC : (q + 1) * 4 * TC])
            return g

        # encoder weights, gate-emission order (r, z on sync; the large
        # bf16 n weights ride the scalar queue in parallel)
        whEr = whpool.tile([128, KH, H], rz_dt, tag="wh")
        nc.sync.dma_start(whEr[:], d_whEr)
        whEz = whpool.tile([128, KH, H], rz_dt, tag="wh")
        nc.sync.dma_start(whEz[:], d_whEz)
        whEn = whnpool.tile([128, KH, H], BF16, tag="whn")
        nc.scalar.dma_start(whEn[:], d_whEn)
        gi_queue = deque([new_gi_chunk(d_giE, 0), new_gi_chunk(d_giE, 1),
                          new_gi_chunk(d_giD, 0)])
        whDr = whpool.tile([128, KH, H], rz_dt, tag="wh")
        nc.sync.dma_start(whDr[:], d_whDr)
        whDz = whpool.tile([128, KH, H], rz_dt, tag="wh")
        nc.sync.dma_start(whDz[:], d_whDz)
        whDn = whnpool.tile([128, KH, H], BF16, tag="whn")
        nc.scalar.dma_start(whDn[:], d_whDn)
        # resident vocab slabs
        slabs = []
        for c in range(R_RES):
            sl = spool.tile([128, KH, VCH], BF16, tag="slab")
            nc.sync.dma_start(sl[:], d_ow[c])
            slabs.append(sl)

        ghnE = ghnD = None
        if has_ghn_e:
            ghnE = cpool.tile([128, MG], F32, tag="ghnE")
            nc.sync.dma_start(ghnE[:], d_ghn_e)
        if has_ghn_d:
            ghnD = cpool.tile([128, MG], F32, tag="ghnD")
            nc.sync.dma_start(ghnD[:], d_ghn_d)
        ones_row = outb_sb = None
        if has_outb:
            ones_row = cpool.tile([1, 128], BF16, tag="ones")
            nc.vector.memset(ones_row[:], 1.0)
            outb_sb = cpool.tile([1, VPAD], BF16, tag="outb")
            nc.sync.dma_start(outb_sb[:], d_outb)

        h_b0 = hpool.tile([128, KH, TC], BF16, tag="hbE")
        nc.vector.tensor_copy(h_b0[:], h_f0[:])

        # per-step-block hidden-state history tiles (bf16, written by the
        # gate chain directly; stationary operand of the projection)
        hst_t = []
        for b in range(F // 4):
            ht = hstpool.tile([128, KH, 4, TC], BF16, tag="hst")
            hst_t.append(ht)

        def gru_step(gis, col0, whz, whr, whn, hprev_of_k, hprev_full,
                     hb_out, ghn):
            """One GRU step (bf16 h recurrence).  gis: gi table tile; cols
            [col0, col0+TC).  hprev_of_k(k) -> [128, TC] bf16 moving AP,
            hprev_full: [128, KH, TC]-shaped bf16 AP of the previous h.
            hb_out: [128, KH, TC]-shaped bf16 output AP (may be strided)."""
            # r/z chains and n chains accumulate into SEPARATE psum tiles
            # (distinct banks): rz_pre's dependency then ends at the last
            # z-chain matmul (~2/3 into the gh phase), so the r/z sigmoid
            # runs DURING the n chains and only the short n tail follows
            # the last matmul.
            MH = MG // 2
            ps = ps_gh.tile([128, 2, MG, TC], F32, tag="rz")
            psn_lo = ps_gh.tile([128, MH, TC], F32, tag="n_lo", bufs=1,
                                padded_shape=[128, MH, 128])
            psn_hi = ps_gh.tile([128, MH, TC], F32, tag="n_hi", bufs=1,
                                padded_shape=[128, MH, 128])
            for g, wh in ((0, whr), (1, whz)):
                for j in range(MG):
                    m = j * 128
                    for k in range(KH):
                        nc.tensor.matmul(
                            ps[:, g, j, :],
                            wh[:, k, m : m + 128],
                            hprev_of_k(k),
                            start=(k == 0),
                            stop=(k == KH - 1),
                        )
            for j in range(MG):
                m = j * 128
                psn = psn_lo[:, j, :] if j < MH else psn_hi[:, j - MH, :]
                for k in range(KH):
                    nc.tensor.matmul(
                        psn,
                        whn[:, k, m : m + 128],
                        hprev_of_k(k),
                        start=(k == 0),
                        stop=(k == KH - 1),
                    )
            gi_rz = gis[:, 0:2, :, col0 : col0 + TC]
            gi_n = gis[:, 2, :, col0 : col0 + TC]

            rz_pre = gpool.tile([128, 2, MG, TC], F32, tag="rz_pre")
            nc.vector.tensor_add(rz_pre[:], gi_rz, ps[:])
            rz = gpool.tile([128, 2, MG, TC], F32, tag="rz")
            nc.scalar.activation(rz[:], rz_pre[:], AF.Sigmoid, scale=rz_scale)
            r, z = rz[:, 0], rz[:, 1]
            zh = gpool.tile([128, MG, TC], F32, tag="zh")
            nc.vector.tensor_mul(zh[:], z, hprev_full)
            omz = gpool.tile([128, MG, TC], F32, tag="omz")
            nc.vector.tensor_scalar(
                omz[:], z, -1.0, 1.0,
                mybir.AluOpType.mult, mybir.AluOpType.add,
            )

            # n path split in m-halves: the lo tail runs during the hi
            # n-gate matmuls, so only ~2us of hi tail trails the last MM
            def n_tail(half, psn, ghn_j0):
                sl = slice(ghn_j0, ghn_j0 + MH)
                if ghn is not None:
                    ghn_sb = gpool.tile([128, MH, TC], F32, tag=f"ghn_{half}")
                    for j in range(MH):
                        nc.scalar.activation(
                            ghn_sb[:, j, :], psn[:, j, :], AF.Identity,
                            bias=ghn[:, ghn_j0 + j : ghn_j0 + j + 1],
                        )
                    n_src = ghn_sb[:]
                else:
                    n_src = psn[:]
                rgh = gpool.tile([128, MH, TC], F32, tag=f"rgh_{half}")
                nc.vector.tensor_mul(rgh[:], r[:, sl, :], n_src)
                n_pre = gpool.tile([128, MH, TC], F32, tag=f"npre_{half}")
                nc.vector.tensor_add(n_pre[:], rgh[:], gi_n[:, sl, :])
                n = gpool.tile([128, MH, TC], F32, tag=f"n_{half}")
                nc.scalar.activation(n[:], n_pre[:], AF.Tanh)
                t1 = gpool.tile([128, MH, TC], F32, tag=f"t1_{half}")
                nc.vector.tensor_mul(t1[:], omz[:, sl, :], n[:])
                nc.vector.tensor_add(hb_out[:, sl, :], t1[:], zh[:, sl, :])

            n_tail("lo", psn_lo, 0)
            n_tail("hi", psn_hi, MH)

        def scores_mm(sb, c, slab):
            """Matmuls for step block sb x vocab chunk c; the psum->SBUF
            copy + store are deferred (run them after the gate chain so
            they never sit ahead of the chain ACTs in the engine FIFO)."""
            ps = ps_sc.tile([128, VCH], F32, tag="sc")
            for k in range(KH):
                nc.tensor.matmul(
                    ps[:],
                    hst_t[sb][:, k, :, :],
                    slab[:, k, :],
                    start=(k == 0),
                    stop=False if has_outb else (k == KH - 1),
                )
            if has_outb:
                nc.tensor.matmul(
                    ps[:], ones_row[:], outb_sb[:, c * VCH : (c + 1) * VCH],
                    start=False, stop=True,
                )
            return (ps, sb, c)

        store_q = [0]

        def scores_flush(item):
            ps, sb, c = item
            ncols = min(VCH, V - c * VCH)
            st = stpool.tile([128, VCH], F32, tag="stg")
            nc.scalar.copy(st[:], ps[:])
            # alternate the store between the scalar and sync DMA queues:
            # a single queue can't drain one 256KB store per pair and the
            # staging-ring WAR then stalls the ACT copies (and the PE)
            eng = nc.scalar if store_q[0] % 2 == 0 else nc.sync
            store_q[0] += 1
            eng.dma_start(
                d_scores[128 * sb : 128 * (sb + 1), c * VCH : c * VCH + ncols],
                st[:, :ncols],
            )

        def scores_block(sb, c, slab):
            scores_flush(scores_mm(sb, c, slab))

        # Each step gets a strictly-increasing logical-time floor so the
        # static per-engine instruction order exactly follows the step
        # structure (the cost-model list scheduler otherwise interleaves
        # projection work into the wrong slots); runtime execution still
        # overlaps freely via the dependency semaphores.
        step_ms = [0]

        def next_floor():
            step_ms[0] += 1000
            return tc.tile_wait_until(step_ms[0])

        # PE warm-up during the initial DMA wait: dependency-free matmuls
        # on a zeroed tile flip the HAM clock gate to 8/8 (~3.4us of
        # sustained activity) before the first real step
        warm = cpool.tile([128, 256], BF16, tag="warm")
        nc.vector.memset(warm[:], 0.0)
        ps_w = ps_sc.tile([128, VCH], F32, tag="sc")
        for _ in range(24):
            nc.tensor.matmul(ps_w[:, :128], warm[:, 0:128], warm[:, 128:256],
                             start=True, stop=True)

        # ---- encoder ----
        hb_prev = h_b0
        for s in range(C):
            with next_floor():
                if s % 4 == 0 and s > 0:
                    gi_queue.popleft()
                gi_cur = gi_queue[0]
                if s == 4:
                    gi_queue.append(new_gi_chunk(d_giD, 1))
                hb_new = hpool.tile([128, KH, TC], BF16, tag="hbE")
                hp = hb_prev
                gru_step(gi_cur, (s % 4) * TC, whEz, whEr, whEn,
                         lambda k: hp[:, k, :], hp[:], hb_new[:], ghnE)
                hb_prev = hb_new

        # ---- decoder with interleaved projection ----
        pending = deque()
        deferred = []
        for s in range(F):
            b, i = s // 4, s % 4
            with next_floor():
                if i == 0:
                    gi_queue.popleft()  # s==0 drops the last encoder chunk
                gi_cur = gi_queue[0]
                if i == 0 and b + 2 < F // 4:
                    gi_queue.append(new_gi_chunk(d_giD, b + 2))
                # flush the previous step's projection psums now: the
                # copies schedule into the ACT-idle gh matmul phase, never
                # between the gate-chain activations
                for it in deferred:
                    scores_flush(it)
                deferred = []
                if s == 0:
                    hp = hb_prev
                    hprev_of_k = lambda k: hp[:, k, :]
                    hprev_full = hp[:]
                else:
                    pb, pi = (s - 1) // 4, (s - 1) % 4
                    hprev_of_k = lambda k, pb=pb, pi=pi: hst_t[pb][:, k, pi, :]
                    hprev_full = hst_t[pb][:, :, pi, :]
                gru_step(gi_cur, i * TC, whDz, whDr, whDn,
                         hprev_of_k, hprev_full, hst_t[b][:, :, i, :], ghnD)
                for _ in range(DOSE_SCHED[i]):
                    if pending:
                        sb, c = pending.popleft()
                        deferred.append(scores_mm(sb, c, slabs[c]))
                if i == 3 and b < F // 4 - 1:
                    pending.extend((b, c) for c in range(R_RES))
        with next_floor():
            for it in deferred:
                scores_flush(it)
            while pending:
                sb, c = pending.popleft()
                scores_block(sb, c, slabs[c])

        # ---- tail: last step-block for resident chunks, then the
        # non-resident chunks (streamed) for all step blocks ----
        stream_tiles = {}

        def prefetch_slab(c):
            if c < NV:
                t_ = sspool.tile([128, KH, VCH], BF16, tag="sslab")
                nc.sync.dma_start(t_[:], d_ow[c])
                stream_tiles[c] = t_

        with next_floor():
            prefetch_slab(R_RES)
            prefetch_slab(R_RES + 1)
            for c in range(R_RES):
                scores_block(F // 4 - 1, c, slabs[c])
        for c in range(R_RES, NV):
            with next_floor():
                sl = stream_tiles.pop(c)
                for sb in range(F // 4):
                    scores_block(sb, c, sl)
                # ring slot of chunk c is fully read now; queue the DMA
                # that reuses it (lands ~one chunk ahead of its use)
                prefetch_slab(c + 2)

    nc.compile()
    return nc


def _prep_inputs(token_ctx, char_emb_w, enc_W_ih, enc_W_hh, enc_b_ih, enc_b_hh,
                 dec_W_ih, dec_W_hh, dec_b_ih, dec_b_hh, out_W, out_b,
                 in_sent_token_chars, out_chars):
    """Host-side sharding/layout prep. Returns (in_maps, flags, fixup_info)."""
    tcarr = np.asarray(in_sent_token_chars)[0].reshape(T, C, 3)
    chars = tcarr[:, :, 2]
    xt = tcarr[:, :, 1]
    token_ctx = np.asarray(token_ctx)[0]          # [S, H]
    char_emb_w = np.asarray(char_emb_w, np.float32)  # [V, E]
    out_chars = np.asarray(out_chars)[0]          # [1 + T*F]

    h0 = token_ctx[xt].mean(axis=1).astype(np.float32)      # [T, H]
    gold = out_chars[1 : 1 + T * F].reshape(T, F)
    c0 = out_chars[0]
    c_in = np.concatenate(
        [np.full((T, 1), c0, dtype=gold.dtype), gold[:, :-1]], axis=1
    )                                                        # [T, F]

    # gi tables: G = W_ih @ emb^T (+ foldable biases); r/z rows x RZ_SCALE
    # when their W_hh stationaries are fp8 (sigmoid un-scales).
    def gi_table(W_ih, b_ih, b_hh):
        G = (np.asarray(W_ih, np.float32) @ char_emb_w.T)    # [3H, V]
        b = np.asarray(b_ih, np.float32).copy()
        b[: 2 * H] += np.asarray(b_hh, np.float32)[: 2 * H]
        G += b[:, None]
        if USE_FP8_RZ:
            G[: 2 * H] *= RZ_SCALE
        return G

    GE = gi_table(enc_W_ih, enc_b_ih, enc_b_hh)
    GD = gi_table(dec_W_ih, dec_b_ih, dec_b_hh)

    # per-gate W_hh lhsT layouts
    def whh_gates(W_hh):
        W_hh = np.asarray(W_hh, np.float32)
        outs = []
        for g in range(3):
            w = _to_lhsT_layout(W_hh[g * H : (g + 1) * H])
            if g < 2 and USE_FP8_RZ:
                outs.append((w * RZ_SCALE).astype(npfp8))
            else:
                outs.append(w.astype(npbf16))
        return outs

    whEr_, whEz_, whEn_ = whh_gates(enc_W_hh)
    whDr_, whDz_, whDn_ = whh_gates(dec_W_hh)

    def ghn_layout(b_hh):
        ghn = np.asarray(b_hh, np.float32)[2 * H :]
        return (np.ascontiguousarray(ghn.reshape(MG, 128).T).astype(np.float32),
                bool(np.any(ghn)))

    ghnE_, has_ghn_e = ghn_layout(enc_b_hh)
    ghnD_, has_ghn_d = ghn_layout(dec_b_hh)

    owpad = np.zeros((VPAD, H), np.float32)
    owpad[:V] = np.asarray(out_W)
    owT = np.ascontiguousarray(
        owpad.reshape(NV, VCH, KH, 128).transpose(0, 3, 2, 1)
    ).astype(npbf16)                                          # [NV,128,KH,VCH]
    out_b = np.asarray(out_b)
    has_outb = bool(np.any(out_b))
    outb_pad = np.zeros((1, VPAD), npbf16)
    outb_pad[0, :V] = out_b.astype(npbf16)

    flags = (has_ghn_e, has_ghn_d, has_outb)

    in_maps = []
    for ci in range(NCORES):
        sl = slice(ci * TC, (ci + 1) * TC)
        h0T = np.ascontiguousarray(
            h0[sl].T.reshape(KH, 128, TC).transpose(1, 0, 2)
        )
        # enc gi: ts = c*TC + t (step-major)
        colsE = chars[sl].T.reshape(-1)
        giE = np.ascontiguousarray(
            GE[:, colsE].reshape(3, MG, 128, C * TC).transpose(2, 0, 1, 3)
        ).astype(npbf16)
        # dec gi: ts = s*TC + t (step-major)
        colsD = c_in[sl].T.reshape(-1)
        giD = np.ascontiguousarray(
            GD[:, colsD].reshape(3, MG, 128, TS).transpose(2, 0, 1, 3)
        ).astype(npbf16)
        m = {
            "h0T": h0T, "giE": giE, "giD": giD,
            "whEr": whEr_, "whEz": whEz_, "whEn": whEn_,
            "whDr": whDr_, "whDz": whDz_, "whDn": whDn_,
            "owT": owT,
        }
        if has_ghn_e: m["ghnE"] = ghnE_
        if has_ghn_d: m["ghnD"] = ghnD_
        if has_outb: m["outb"] = outb_pad
        in_maps.append(m)

    return in_maps, flags, (gold, c0)


def _eos_fixup(scores, gold, c0):
    """Apply the reference's EOS freeze/pad semantics on the host.
    scores: [T, F, V] (modified in place)."""
    if c0 != EOS and not np.any(gold == EOS):
        return scores
    done0 = c0 == EOS
    for t in range(T):
        hits = np.nonzero(gold[t] == EOS)[0]
        if done0:
            first_done = 0
        elif len(hits):
            first_done = int(hits[0]) + 1
        else:
            continue
        if first_done == 0:
            scores[t, :, :] = 0.0
        elif first_done < F:
            scores[t, first_done:, :] = scores[t, first_done - 1, :]
    return scores


def kernel(**inputs) -> np.ndarray:
    assert int(inputs["max_tokens"]) == T
    assert int(inputs["max_form_len"]) == F
    assert int(inputs["use_teacher_forcing"]) == 1

    in_maps, flags, (gold, c0) = _prep_inputs(
        inputs["token_ctx"], inputs["char_emb_w"],
        inputs["enc_W_ih"], inputs["enc_W_hh"], inputs["enc_b_ih"], inputs["enc_b_hh"],
        inputs["dec_W_ih"], inputs["dec_W_hh"], inputs["dec_b_ih"], inputs["dec_b_hh"],
        inputs["out_W"], inputs["out_b"],
        inputs["in_sent_token_chars"], inputs["out_chars"],
    )

    if flags not in _CACHE:
        _CACHE[flags] = _build_program(flags)
    nc = _CACHE[flags]

    trace = bool(_RUN_OPTS.get("trace"))
    res = run_bass_kernel_spmd(
        nc, in_maps, core_ids=list(range(NCORES)), trace=trace,
        **_RUN_OPTS.get("kwargs", {}),
    )
    _RUN_OPTS["last_result"] = res

    # device rows are step-major per core; reorder to token-major
    slabs = [
        res.results[ci]["scores"].reshape(F, TC, V).transpose(1, 0, 2)
        for ci in range(NCORES)
    ]
    out = np.concatenate(slabs, axis=0)  # [T, F, V]
    out = _eos_fixup(out, gold, c0)
    return np.ascontiguousarray(out.reshape(1, T * F, V))


# knobs used by test.py (harness just calls kernel())
_RUN_OPTS = {"trace": False, "kwargs": {}}
